# revision 24
# baseline (speedup 1.0000x reference)
"""v11: bf16 MHA, column-tiled attnV + off-PE softmax denominators.

Design (per core: one batch b = c//2, head-group g = c%2 of E=512 dims):
- Attention pipeline per j-tile: QK^T row-tiled pair (2 heads, concurrent,
  512 cyc) -> exp on ACT ([128,1024] tile, ~1.04us, THE bottleneck) ->
  attnV column-tiled pair (M=64 per head, tile positions (0,0)/(0,64),
  concurrent, 512 cyc) accumulating into one a_ps [128,512] bank.
- Softmax denominators no longer ride the attnV stationary (v10's 65th
  column forced serial M=65 attnV): DVE accumulates E_sum += e_j (bf16,
  errors average out in the k-sum), then a column-tiled ones-matmul pair
  computes Z_A/Z_B rows, reciprocal on DVE, and a column-tiled K=1
  broadcast pair expands 1/Z to 128 partitions. Normalization is then a
  single [128,512] DVE multiply writing aT in its natural layout.
- All other PE work (Q/K/V/O projections) is pump-filler between
  attention steps, due-key forced ahead of need; ACT runs exp only.
- PSUM: s_ps double-buffer (4 banks) + a_ps [128,512] pair (2) + work
  pool (2) = 8 banks exactly.
- HW-validated: row/col-tiled matmul pairs run concurrently on the PE
  (ubench: ~200ns per pair = one full matmul); the serial-charging sim
  overstates PE time for tiled pairs by ~110us.
CoreSim span 422us (v10) -> ~300us sim / ~285us HW expected.
"""

from collections import deque

import numpy as np
import ml_dtypes

import concourse.bass as bass
import concourse.mybir as mybir
import concourse.tile as tile
from concourse import bacc
from concourse.bass_utils import run_bass_kernel_spmd

B, S, D = 4, 2048, 1024
HT, DK = 16, 64
G = 2
NCORES = 8
E = D // G
H = HT // G
EC = E // 128
KD = D // 128
SM = S // 128
SN = S // 512
F32 = mybir.dt.float32
BF16 = mybir.dt.bfloat16
NPBF16 = ml_dtypes.bfloat16
EXP = mybir.ActivationFunctionType.Exp

REPS = 1


def _build_mha_nc(reps=1):
    nc = bacc.Bacc("TRN2", target_bir_lowering=False, debug=False)

    xq = nc.dram_tensor("xq_t", [D, S], BF16, kind="ExternalInput")
    xk = nc.dram_tensor("xk_t", [D, S], BF16, kind="ExternalInput")
    xv = nc.dram_tensor("xv_t", [D, S], BF16, kind="ExternalInput")
    wq = nc.dram_tensor("wq_t", [D, E], BF16, kind="ExternalInput")
    wk = nc.dram_tensor("wk_t", [D, E], BF16, kind="ExternalInput")
    wv = nc.dram_tensor("wv_t", [D, E], BF16, kind="ExternalInput")
    wo = nc.dram_tensor("wo_t", [E, D], BF16, kind="ExternalInput")
    bq = nc.dram_tensor("b_q", [E], BF16, kind="ExternalInput")
    bk = nc.dram_tensor("b_k", [E], BF16, kind="ExternalInput")
    out = nc.dram_tensor("out", [S, D], F32, kind="ExternalOutput")

    with tile.TileContext(nc) as tc:
        for _ in range(reps):
            _mha_body(tc, xq, xk, xv, wq, wk, wv, wo, bq, bk, out)
    nc.compile()
    return nc


class _Pump:
    """Filler-work queue: units are generators yielding ~matmul-sized chunks."""

    def __init__(self):
        self.q = deque()

    def push(self, due, gen):
        self.q.append([due, gen])

    def pump(self, budget, key=None):
        while budget > 0 and self.q:
            # due units first (anywhere in the queue), then FIFO front
            pick = 0
            if key is not None:
                for i, (due, g) in enumerate(self.q):
                    if due is not None and due <= key:
                        pick = i
                        break
            due, g = self.q[pick]
            try:
                budget -= next(g)
            except StopIteration:
                del self.q[pick]

    def has_due(self, key):
        for due, _ in self.q:
            if due is not None and due <= key:
                return True
        return False

    def drain_due(self, key):
        kept = deque()
        while self.q:
            due, g = self.q.popleft()
            if due is not None and due <= key:
                for _ in g:
                    pass
            else:
                kept.append([due, g])
        self.q = kept

    def drain_all(self):
        while self.q:
            _, g = self.q.popleft()
            for _ in g:
                pass


def _mha_body(tc, xq, xk, xv, wq, wk, wv, wo, bq, bk, out):
    nc = tc.nc
    from contextlib import ExitStack

    with ExitStack() as ctx:
        singles = ctx.enter_context(tc.tile_pool(name="singles", bufs=1))
        persist = ctx.enter_context(tc.tile_pool(name="persist", bufs=1))
        ps_s = ctx.enter_context(tc.tile_pool(name="ps_s", bufs=2, space="PSUM"))
        ps_a = ctx.enter_context(tc.tile_pool(name="ps_a", bufs=2, space="PSUM"))
        ps_w = ctx.enter_context(tc.tile_pool(name="ps_w", bufs=2, space="PSUM"))

        ones_col = singles.tile([128, 1], BF16)
        nc.vector.memset(ones_col, 1.0)
        ones_row = singles.tile([128, 64], BF16)
        nc.vector.memset(ones_row, 1.0)
        ones_512 = singles.tile([1, 512], BF16)
        nc.vector.memset(ones_512, 1.0)
        bqr = singles.tile([1, 512], BF16)
        bkr = singles.tile([1, 512], BF16)

        qT = [persist.tile([128, S], BF16, name=f"qT{c}") for c in range(EC)]
        kT = [persist.tile([128, S], BF16, name=f"kT{c}") for c in range(EC)]
        v_sb = [persist.tile([128, H * DK], BF16, name=f"v{s}") for s in range(SM)]
        aT = [persist.tile([128, S], BF16, name=f"aT{p}") for p in range(EC)]
        wo_sb = []

        x_pool = ctx.enter_context(tc.tile_pool(name="x_sb", bufs=1))
        xq_sb = [x_pool.tile([128, S], BF16, name=f"xq{d}") for d in range(KD)]
        wq_sb = [x_pool.tile([128, E], BF16, name=f"wq{d}") for d in range(KD)]
        e_pool = ctx.enter_context(tc.tile_pool(name="e_sb", bufs=3))
        acc_pool = ctx.enter_context(tc.tile_pool(name="acc_sb", bufs=2))
        acc2_pool = ctx.enter_context(tc.tile_pool(name="acc2_sb", bufs=2))
        acc3_pool = ctx.enter_context(tc.tile_pool(name="acc3_sb", bufs=2))
        nrm_pool = ctx.enter_context(tc.tile_pool(name="nrm", bufs=1))
        late_pools = {}

        kv_ctx = ExitStack()
        kv_pool = kv_ctx.enter_context(tc.tile_pool(name="kv_sb", bufs=1))
        xk_sb = [kv_pool.tile([128, S], BF16, name=f"xk{d}") for d in range(KD)]
        xv_sb = [kv_pool.tile([128, S], BF16, name=f"xv{d}") for d in range(KD)]
        wk_sb = [kv_pool.tile([128, E], BF16, name=f"wk{d}") for d in range(KD)]
        wv_sb = [kv_pool.tile([128, E], BF16, name=f"wv{d}") for d in range(KD)]

        # --- input staging ---
        # Two parallel DMA paths: sync->HWDGE (~205GB/s, 625ns/instr hold)
        # and gpsimd->SWDGE (~123GB/s, ~1us Pool hold). One global
        # need-ordered chunk list, dispatched 2:1 sync:gpsimd. Nothing on
        # the scalar queue (ACT runs exp only); evac copies are on DVE.
        chunks = []

        def _add(sb_ap, dram_ap):
            chunks.append((sb_ap, dram_ap))

        for d in range(KD):
            _add(wk_sb[d], wk[d * 128 : (d + 1) * 128, :])
            _add(xk_sb[d][:, 0:512], xk[d * 128 : (d + 1) * 128, 0:512])
        for d in range(KD):
            _add(wq_sb[d], wq[d * 128 : (d + 1) * 128, :])
            _add(xq_sb[d][:, 0:512], xq[d * 128 : (d + 1) * 128, 0:512])
        for c in range(EC):
            _add(bkr[0:1, c * 128 : (c + 1) * 128], bk[c * 128 : (c + 1) * 128])
            _add(bqr[0:1, c * 128 : (c + 1) * 128], bq[c * 128 : (c + 1) * 128])
        for d in range(KD):
            _add(wv_sb[d], wv[d * 128 : (d + 1) * 128, :])
        for sc in range(SN):
            for d in range(KD):
                _add(
                    xv_sb[d][:, sc * 512 : (sc + 1) * 512],
                    xv[d * 128 : (d + 1) * 128, sc * 512 : (sc + 1) * 512],
                )
            if sc + 1 < SN:
                for d in range(KD):
                    _add(
                        xk_sb[d][:, (sc + 1) * 512 : (sc + 2) * 512],
                        xk[d * 128 : (d + 1) * 128, (sc + 1) * 512 : (sc + 2) * 512],
                    )
        for sc in range(1, SN):
            for d in range(KD):
                _add(
                    xq_sb[d][:, sc * 512 : (sc + 1) * 512],
                    xq[d * 128 : (d + 1) * 128, sc * 512 : (sc + 1) * 512],
                )
        for i, (sb_ap, dram_ap) in enumerate(chunks):
            q = nc.gpsimd if i % 3 == 2 else nc.sync
            q.dma_start(out=sb_ap, in_=dram_ap)
        def kproj_unit(p, s):
            p_ps = ps_w.tile([128, 512], F32, name="p_ps", tag="w")
            for d in range(KD):
                nc.tensor.matmul(
                    p_ps,
                    wk_sb[d][:, p * 128 : (p + 1) * 128],
                    xk_sb[d][:, s * 512 : (s + 1) * 512],
                    start=(d == 0),
                    stop=False,
                )
                yield 1
            nc.tensor.matmul(
                p_ps, bkr[0:1, p * 128 : (p + 1) * 128], ones_512[0:1, :],
                start=False, stop=True
            )
            yield 1
            nc.vector.tensor_copy(kT[p][:, s * 512 : (s + 1) * 512], p_ps)
            yield 0

        def qproj_unit(p, ic):
            p_ps = ps_w.tile([128, 512], F32, name="p_ps", tag="w")
            for d in range(KD):
                nc.tensor.matmul(
                    p_ps,
                    wq_sb[d][:, p * 128 : (p + 1) * 128],
                    xq_sb[d][:, ic * 512 : (ic + 1) * 512],
                    start=(d == 0),
                    stop=False,
                )
                yield 1
            nc.tensor.matmul(
                p_ps, bqr[0:1, p * 128 : (p + 1) * 128], ones_512[0:1, :],
                start=False, stop=True
            )
            yield 1
            nc.vector.tensor_copy(qT[p][:, ic * 512 : (ic + 1) * 512], p_ps)
            yield 0

        def outproj_unit(ic, s, e2):
            o_ps = ps_w.tile([128, 512], F32, name="o_ps", tag="w")
            for d in range(EC):
                nc.tensor.matmul(
                    o_ps,
                    aT[d][:, s * 128 : (s + 1) * 128],
                    wo_sb[d][:, e2 * 512 : (e2 + 1) * 512],
                    start=(d == 0),
                    stop=(d == EC - 1),
                )
                yield 1
            o_sb = late_pools["out"].tile([128, 512], F32, name="o_sb", tag="o")
            nc.vector.tensor_copy(o_sb, o_ps)
            nc.sync.dma_start(
                out=out[s * 128 : (s + 1) * 128, e2 * 512 : (e2 + 1) * 512],
                in_=o_sb,
            )
            yield 0

        def vproj_pp(half, j):
            # project V columns for heads {4*half..4*half+3}, key-tile j
            v_ps = ps_w.tile([128, 256], F32, name="v_ps", tag="w")
            for d in range(KD):
                nc.tensor.matmul(
                    v_ps,
                    xv_sb[d][:, j * 128 : (j + 1) * 128],
                    wv_sb[d][:, half * 256 : (half + 1) * 256],
                    start=(d == 0),
                    stop=(d == KD - 1),
                )
                yield 1
            nc.vector.tensor_copy(v_sb[j][:, half * 256 : (half + 1) * 256], v_ps)
            yield 0

        pump = _Pump()

        def make_tail(p, ic, a_ps, acc_ev, acc_od, acc_p, e_last):
            # trailing work of block (p, ic), split in two: tail_pe at j==0
            # of the next block (last attnV pair + final odd-chain add, so
            # e_last's buffer recycles quickly), tail_fin at j==2 (E_sum
            # merge + denominators + normalization, off the e-recycle path).
            i0 = ic * 512
            h0, h1 = 2 * p, 2 * p + 1

            def tail_pe():
                nc.tensor.matmul(
                    a_ps[0:64, :],
                    v_sb[SM - 1][:, h0 * DK : (h0 + 1) * DK],
                    e_last[:, 0:512],
                    start=False,
                    stop=True,
                )
                nc.tensor.matmul(
                    a_ps[64:128, :],
                    v_sb[SM - 1][:, h1 * DK : (h1 + 1) * DK],
                    e_last[:, 512:1024],
                    start=False,
                    stop=True,
                )
                with nc.allow_low_precision(
                    reason="E_sum bf16 accumulation; Z error averages out"
                ):
                    nc.vector.tensor_add(acc_od, acc_od, e_last)

            def tail_fin():
                with nc.allow_low_precision(
                    reason="E_sum bf16 accumulation; Z error averages out"
                ):
                    nc.vector.tensor_add(acc_ev, acc_ev, acc_od)
                    if acc_p is not None:
                        nc.vector.tensor_add(acc_ev, acc_ev, acc_p)
                acc = acc_ev
                # Z rows via column-tiled ones-matmul pair on E_sum
                z_ps = ps_w.tile([128, 512], F32, name="z_ps", tag="w")
                nc.tensor.matmul(
                    z_ps[0:1, :], ones_col, acc[:, 0:512], start=True, stop=True
                )
                nc.tensor.matmul(
                    z_ps[64:65, :], ones_col, acc[:, 512:1024],
                    start=True, stop=True,
                )
                rec = nrm_pool.tile([128, 512], BF16, name="rec", tag="rec")
                with nc.allow_low_precision(
                    reason="softmax denom reciprocal; bf16 err ~4e-3 verified"
                ):
                    nc.vector.reciprocal(rec[0:1, :], z_ps[0:1, :])
                    nc.vector.reciprocal(rec[64:65, :], z_ps[64:65, :])
                # broadcast 1/Z rows to 64 partitions each (K=1 pair)
                bc_ps = z_ps
                nc.tensor.matmul(
                    bc_ps[0:64, :], ones_row[0:1, :], rec[0:1, :],
                    start=True, stop=True,
                )
                nc.tensor.matmul(
                    bc_ps[64:128, :], ones_row[64:65, :], rec[64:65, :],
                    start=True, stop=True,
                )
                bc_sb = nrm_pool.tile([128, 512], BF16, name="bc_sb", tag="bc")
                with nc.allow_low_precision(
                    reason="softmax denom broadcast; bf16 err ~4e-3 verified"
                ):
                    nc.vector.tensor_copy(bc_sb, bc_ps)
                    nc.vector.tensor_mul(aT[p][:, i0 : i0 + 512], a_ps, bc_sb)

            return tail_pe, tail_fin

        def block(p, ic, prev_tail):
            i0 = ic * 512
            h0, h1 = 2 * p, 2 * p + 1
            pump.drain_due((p, ic))
            a_ps = ps_a.tile([128, 512], F32, name="a_ps", tag="a")
            acc_ev = acc_pool.tile([128, 1024], BF16, name="acc_ev", tag="acc")
            acc_od = acc2_pool.tile([128, 1024], BF16, name="acc_od", tag="acc2")
            acc_p = (
                acc3_pool.tile([128, 1024], BF16, name="acc_p", tag="acc3")
                if p > 0
                else None
            )
            e_tiles = {}
            for j in range(SM):
                s_ps = ps_s.tile([128, 1024], F32, name="s_ps", tag="s")
                with tc.high_priority():
                    nc.tensor.matmul(
                        s_ps[:, 0:512],
                        kT[p][0:64, j * 128 : (j + 1) * 128],
                        qT[p][0:64, i0 : i0 + 512],
                        start=True,
                        stop=True,
                    )
                    nc.tensor.matmul(
                        s_ps[:, 512:1024],
                        kT[p][64:128, j * 128 : (j + 1) * 128],
                        qT[p][64:128, i0 : i0 + 512],
                        start=True,
                        stop=True,
                    )
                ep = late_pools.get("e2", e_pool)
                e0 = ep.tile([128, 1024], BF16, name="e0", tag="e")
                nc.scalar.activation(e0, s_ps, EXP, scale=0.125)
                e_tiles[j] = e0
                if j == 0 and prev_tail is not None:
                    prev_tail[0]()
                if j == 4 and prev_tail is not None:
                    prev_tail[1]()
                    for due, gen in pending:
                        pump.push(due, gen)
                    del pending[:]
                if j > 0:
                    ej = e_tiles.pop(j - 1)
                    nc.tensor.matmul(
                        a_ps[0:64, :],
                        v_sb[j - 1][:, h0 * DK : (h0 + 1) * DK],
                        ej[:, 0:512],
                        start=(j - 1 == 0),
                        stop=False,
                    )
                    nc.tensor.matmul(
                        a_ps[64:128, :],
                        v_sb[j - 1][:, h1 * DK : (h1 + 1) * DK],
                        ej[:, 512:1024],
                        start=(j - 1 == 0),
                        stop=False,
                    )
                    jj = j - 1
                    if jj % 2 == 0:
                        eng, acc, first = nc.vector, acc_ev, jj == 0
                    elif acc_p is not None and jj % 4 == 3:
                        eng, acc, first = nc.gpsimd, acc_p, jj == 3
                    else:
                        eng, acc, first = nc.vector, acc_od, jj == 1
                    with nc.allow_low_precision(
                        reason="E_sum bf16 accumulation; Z error averages out"
                    ):
                        if first:
                            eng.tensor_copy(acc, ej)
                        else:
                            eng.tensor_add(acc, acc, ej)
                if p == 0 and ic == 0:
                    for _ in vproj_pp(0, j):
                        pass
                pump.pump(1 if (p == 0 and ic == 0) else (3 if pump.has_due((p, ic)) else 2), (p, ic))
            return make_tail(p, ic, a_ps, acc_ev, acc_od, acc_p, e_tiles.pop(SM - 1))

        # --- prologue: first two kproj chunks + first qproj, serial ---
        for _ in kproj_unit(0, 0):
            pass
        for _ in qproj_unit(0, 0):
            pass
        pump.push((0, -1), kproj_unit(0, 1))
        pump.push((0, -1), kproj_unit(0, 2))
        pump.push((0, -1), kproj_unit(0, 3))
        for ic in range(1, SN):
            pump.push((0, ic - 1.1), qproj_unit(0, ic))

        # --- main loop: ic-fast within head-pair groups ---
        tail = None
        pending = []
        for p in range(EC):
            if p + 1 < EC:
                # next group prereqs, spread across this group
                for sc in range(SN):
                    pump.push((p, sc - 0.5), kproj_unit(p + 1, sc))
                for ic in range(SN):
                    pump.push((p, ic - 0.5), qproj_unit(p + 1, ic))
            if p < 2:
                # vproj for heads 4-7 (pairs 2,3): spread over groups 0-1,
                # after each group's kproj/qproj prereqs in queue order
                for j in range(8 * p, 8 * p + 8):
                    pump.push((p, 2 + (j % 8) / 4.0), vproj_pp(1, j))
            for ic in range(SN):
                tail = block(p, ic, tail)
                if p == EC - 1:
                    for sq in range(4):
                        for e2 in range(2):
                            pending.append(
                                ((3, ic + sq / 4.0),
                                 outproj_unit(ic, ic * 4 + sq, e2))
                            )
            if p == 1:
                # K/V + x staging no longer needed shortly; close after
                # group 2 prereqs are all forced (kproj(3) due keys < (2,*)).
                pass
            if p == 2:
                kv_ctx.close()
                late_pools["e2"] = ctx.enter_context(
                    tc.tile_pool(name="e2_sb", bufs=8)
                )
                late_pools["out"] = ctx.enter_context(
                    tc.tile_pool(name="osb", bufs=4)
                )
                late_pools["wo"] = ctx.enter_context(
                    tc.tile_pool(name="wo_sb", bufs=1)
                )
                for d in range(EC):
                    wo_sb.append(
                        late_pools["wo"].tile([128, D], BF16, name=f"wo{d}")
                    )
                    nc.sync.dma_start(
                        out=wo_sb[d], in_=wo[d * 128 : (d + 1) * 128, :]
                    )
        tail[0]()
        tail[1]()
        for due, gen in pending:
            pump.push(due, gen)
        del pending[:]
        pump.drain_all()


def _prep_in_maps(query, key, value, w_q, b_q, w_k, b_k, w_v, b_v, w_o):
    f32 = np.float32
    in_maps = []
    for c in range(NCORES):
        b, g = c // G, c % G
        sl = slice(g * E, (g + 1) * E)
        in_maps.append(
            {
                "xq_t": np.ascontiguousarray(query[b].T).astype(NPBF16),
                "xk_t": np.ascontiguousarray(key[b].T).astype(NPBF16),
                "xv_t": np.ascontiguousarray(value[b].T).astype(NPBF16),
                "wq_t": np.ascontiguousarray(w_q[sl, :].T).astype(NPBF16),
                "wk_t": np.ascontiguousarray(w_k[sl, :].T).astype(NPBF16),
                "wv_t": np.ascontiguousarray(w_v[sl, :].T).astype(NPBF16),
                "wo_t": np.ascontiguousarray(w_o[:, sl].T).astype(NPBF16),
                "b_q": np.ascontiguousarray(b_q[sl]).astype(NPBF16),
                "b_k": np.ascontiguousarray(b_k[sl]).astype(NPBF16),
            }
        )
    return in_maps


_NC_CACHE = {}


def run(inputs, trace=False, **kw):
    if REPS not in _NC_CACHE:
        _NC_CACHE[REPS] = _build_mha_nc(REPS)
    nc = _NC_CACHE[REPS]
    in_maps = _prep_in_maps(
        inputs["query"], inputs["key"], inputs["value"],
        inputs["w_q"], inputs["b_q"], inputs["w_k"], inputs["b_k"],
        inputs["w_v"], inputs["b_v"], inputs["w_o"],
    )
    res = run_bass_kernel_spmd(nc, in_maps, list(range(NCORES)), trace=trace, **kw)
    bias_vec = (
        np.asarray(inputs["b_o"], dtype=np.float32)
        + np.asarray(inputs["w_o"], dtype=np.float32)
        @ np.asarray(inputs["b_v"], dtype=np.float32)
    )
    full = np.empty((B, S, D), dtype=np.float32)
    for b in range(B):
        full[b] = res.results[2 * b]["out"] + res.results[2 * b + 1]["out"] + bias_vec
    return full, res


def kernel(**inputs):
    full, _ = run(inputs)
    return full


def _make_timed_callable(nc, in_maps):
    import jax
    from jax.sharding import Mesh, PartitionSpec
    from jax.experimental.shard_map import shard_map
    from concourse import bass2jax, mybir as mb

    partition_name = nc.partition_id_tensor.name if nc.partition_id_tensor else None
    in_names, out_names, out_avals, zero_outs = [], [], [], []
    for alloc in nc.m.functions[0].allocations:
        if not isinstance(alloc, mb.MemoryLocationSet):
            continue
        name = alloc.memorylocations[0].name
        if alloc.kind == "ExternalInput":
            if name != partition_name:
                in_names.append(name)
        elif alloc.kind == "ExternalOutput":
            out_names.append(name)
            shape = tuple(alloc.tensor_shape)
            dtype = mb.dt.np(alloc.dtype)
            out_avals.append(jax.core.ShapedArray(shape, dtype))
            zero_outs.append(np.zeros(shape, dtype))
    n_params = len(in_names)
    in_names = in_names + out_names
    if partition_name is not None:
        in_names.append(partition_name)
    donate = tuple(range(n_params, n_params + len(out_names)))

    def _body(*args):
        operands = list(args)
        if partition_name is not None:
            operands.append(bass2jax.partition_id_tensor())
        outs = bass2jax._bass_exec_p.bind(
            *operands,
            out_avals=tuple(out_avals),
            in_names=tuple(in_names),
            out_names=tuple(out_names),
            lowering_input_output_aliases=(),
            sim_require_finite=True,
            sim_require_nnan=True,
            nc=nc,
        )
        return tuple(outs)

    devices = jax.devices()[:NCORES]
    mesh = Mesh(np.asarray(devices).reshape(NCORES), ("core",))
    in_specs = (PartitionSpec("core"),) * (n_params + len(out_names))
    out_specs = (PartitionSpec("core"),) * len(out_names)
    sharded = jax.jit(
        shard_map(_body, mesh=mesh, in_specs=in_specs, out_specs=out_specs,
                  check_rep=False),
        donate_argnums=donate, keep_unused=True,
    )
    concat_in = [
        np.concatenate([in_maps[c][in_names[i]] for c in range(NCORES)], axis=0)
        for i in range(n_params)
    ]
    dev_in = [jax.device_put(a) for a in concat_in]

    def call():
        zeros_dev = [
            jax.device_put(np.zeros((NCORES * z.shape[0], *z.shape[1:]), z.dtype))
            for z in zero_outs
        ]
        jax.block_until_ready(zeros_dev)
        import time

        t0 = time.perf_counter()
        out_arrs = sharded(*dev_in, *zeros_dev)
        jax.block_until_ready(out_arrs)
        dt = time.perf_counter() - t0
        return out_arrs, dt

    def gather(out_arrs):
        return [
            {
                name: np.asarray(out_arrs[i]).reshape(NCORES, *out_avals[i].shape)[c]
                for i, name in enumerate(out_names)
            }
            for c in range(NCORES)
        ]

    return call, gather


def run_timed(inputs, iters=6):
    """Measure device execution via repeated pjrt calls (amortizes RPC)."""
    global REPS
    from concourse import bass2jax

    bass2jax.install_neuronx_cc_hook()
    in_maps = _prep_in_maps(
        inputs["query"], inputs["key"], inputs["value"],
        inputs["w_q"], inputs["b_q"], inputs["w_k"], inputs["b_k"],
        inputs["w_v"], inputs["b_v"], inputs["w_o"],
    )
    if REPS not in _NC_CACHE:
        _NC_CACHE[REPS] = _build_mha_nc(REPS)
    call, gather = _make_timed_callable(_NC_CACHE[REPS], in_maps)
    times = []
    out_arrs = None
    for _ in range(iters):
        out_arrs, dt = call()
        times.append(dt)
    res = gather(out_arrs)
    bias_vec = (
        np.asarray(inputs["b_o"], dtype=np.float32)
        + np.asarray(inputs["w_o"], dtype=np.float32)
        @ np.asarray(inputs["b_v"], dtype=np.float32)
    )
    full = np.empty((B, S, D), dtype=np.float32)
    for b in range(B):
        full[b] = res[2 * b]["out"] + res[2 * b + 1]["out"] + bias_vec
    return full, times


def run_timed_pair(inputs, nrep=25, iters=40):
    """Interleave REPS=1 and REPS=nrep executions so slow wall-clock drift
    cancels in the per-iteration delta. Returns (full_output_r1, deltas_s)
    where deltas[i] = t_rn[i] - t_r1[i]; kernel time ~= median(deltas)/(nrep-1).
    """
    global REPS
    from concourse import bass2jax

    bass2jax.install_neuronx_cc_hook()
    in_maps = _prep_in_maps(
        inputs["query"], inputs["key"], inputs["value"],
        inputs["w_q"], inputs["b_q"], inputs["w_k"], inputs["b_k"],
        inputs["w_v"], inputs["b_v"], inputs["w_o"],
    )
    old = REPS
    try:
        for r in (1, nrep):
            REPS = r
            if r not in _NC_CACHE:
                _NC_CACHE[r] = _build_mha_nc(r)
    finally:
        REPS = old
    call1, gather1 = _make_timed_callable(_NC_CACHE[1], in_maps)
    calln, gathern = _make_timed_callable(_NC_CACHE[nrep], in_maps)
    out1, _ = call1()
    outn, _ = calln()  # compile+warm both
    t1s, tns = [], []
    for _ in range(iters):
        out1, dt1 = call1()
        outn, dtn = calln()
        t1s.append(dt1)
        tns.append(dtn)
    res1, resn = gather1(out1), gathern(outn)
    for c in range(NCORES):
        assert np.allclose(res1[c]["out"], resn[c]["out"], atol=1e-5)
    bias_vec = (
        np.asarray(inputs["b_o"], dtype=np.float32)
        + np.asarray(inputs["w_o"], dtype=np.float32)
        @ np.asarray(inputs["b_v"], dtype=np.float32)
    )
    full = np.empty((B, S, D), dtype=np.float32)
    for b in range(B):
        full[b] = res1[2 * b]["out"] + res1[2 * b + 1]["out"] + bias_vec
    return full, t1s, tns


# revision 25
# speedup vs baseline: 2.6063x; 2.6063x over previous
"""v12: bf16 MHA; column-tiled attnV, off-PE softmax denominators,
ic-fast group schedule.

Design (per core: one batch b = c//2, head-group g = c%2 of E=512 dims):
- Per j-tile: QK^T row-tiled pair (2 heads, concurrent on HW, 512 cyc)
  -> exp on ACT ([128,1024] tile ~1.04us, the throughput anchor) ->
  attnV column-tiled pair (M=64/head at psum partitions 0-63/64-127,
  concurrent, 512 cyc) accumulating into one a_ps [128,512] bank.
- Softmax denominators come from E_sum = sum_j e_j, accumulated on
  DVE/Pool in three bf16 chains (errors average out across the key sum),
  then a column-tiled ones-matmul pair (Z rows), DVE reciprocal, and a
  column-tiled K=1 broadcast pair; normalization is one [128,512] DVE
  multiply into aT's natural layout.
- Block order is ic-fast within head-pair groups: group p needs only
  kT[p]/qT[p]/v[pair p], so projection prereqs for group p+1 spread over
  group p as pump fillers (due-keyed, due-first pump). outproj(ic) units
  are held until the tail_fin that writes aT[3][:,ic] has been emitted
  (emission-order race otherwise).
- Projection bias is applied on the PE via a K=1 ones-row matmul into
  the accumulating psum; psum evacuation copies are DVE (GPSIMD cannot
  access PSUM). Block tails are split: tail_pe (last attnV pair + final
  odd-chain add) at j==0 of the next block, tail_fin (merge + Z + bc +
  normalize) at j==4, keeping the e-pool recycle path clear.
- Input staging: one need-ordered chunk list over two parallel DMA
  paths - sync->HWDGE (~205GB/s, 625ns/instr hold) and gpsimd->SWDGE
  (~123GB/s); nothing on the ACT queue.
- PSUM: s_ps double-buffer (4 banks) + a_ps pair (2) + work pool (2)
  = 8 banks; bc_ps reuses z_ps's bank.
- HW-validated: row- and column-tiled matmul pairs execute concurrently
  (~200ns/pair, ubench); serial-charging sim overstates PE by ~110us.
HW rel_rms 6.815e-3. hw-true sim span 384us (v10: ~420); measured
per-rep marginal is protocol-dependent (axon wall-clock noise + device
throttling beyond ~10ms sustained): ~170-250us burst, ~530-630us
sustained at REPS>=51; v10 measures equal-or-slower under every
same-conditions protocol.
"""

from collections import deque

import numpy as np
import ml_dtypes

import concourse.bass as bass
import concourse.mybir as mybir
import concourse.tile as tile
from concourse import bacc
from concourse.bass_utils import run_bass_kernel_spmd

B, S, D = 4, 2048, 1024
HT, DK = 16, 64
G = 2
NCORES = 8
E = D // G
H = HT // G
EC = E // 128
KD = D // 128
SM = S // 128
SN = S // 512
F32 = mybir.dt.float32
BF16 = mybir.dt.bfloat16
NPBF16 = ml_dtypes.bfloat16
EXP = mybir.ActivationFunctionType.Exp

REPS = 1


def _build_mha_nc(reps=1):
    nc = bacc.Bacc("TRN2", target_bir_lowering=False, debug=False)

    xq = nc.dram_tensor("xq_t", [D, S], BF16, kind="ExternalInput")
    xk = nc.dram_tensor("xk_t", [D, S], BF16, kind="ExternalInput")
    xv = nc.dram_tensor("xv_t", [D, S], BF16, kind="ExternalInput")
    wq = nc.dram_tensor("wq_t", [D, E], BF16, kind="ExternalInput")
    wk = nc.dram_tensor("wk_t", [D, E], BF16, kind="ExternalInput")
    wv = nc.dram_tensor("wv_t", [D, E], BF16, kind="ExternalInput")
    wo = nc.dram_tensor("wo_t", [E, D], BF16, kind="ExternalInput")
    bq = nc.dram_tensor("b_q", [E], BF16, kind="ExternalInput")
    bk = nc.dram_tensor("b_k", [E], BF16, kind="ExternalInput")
    out = nc.dram_tensor("out", [S, D], F32, kind="ExternalOutput")

    with tile.TileContext(nc) as tc:
        for _ in range(reps):
            _mha_body(tc, xq, xk, xv, wq, wk, wv, wo, bq, bk, out)
    nc.compile()
    return nc


class _Pump:
    """Filler-work queue: units are generators yielding ~matmul-sized chunks."""

    def __init__(self):
        self.q = deque()

    def push(self, due, gen):
        self.q.append([due, gen])

    def pump(self, budget, key=None):
        while budget > 0 and self.q:
            # due units first (anywhere in the queue), then FIFO front
            pick = 0
            if key is not None:
                for i, (due, g) in enumerate(self.q):
                    if due is not None and due <= key:
                        pick = i
                        break
            due, g = self.q[pick]
            try:
                budget -= next(g)
            except StopIteration:
                del self.q[pick]

    def has_due(self, key):
        for due, _ in self.q:
            if due is not None and due <= key:
                return True
        return False

    def drain_due(self, key):
        kept = deque()
        while self.q:
            due, g = self.q.popleft()
            if due is not None and due <= key:
                for _ in g:
                    pass
            else:
                kept.append([due, g])
        self.q = kept

    def drain_all(self):
        while self.q:
            _, g = self.q.popleft()
            for _ in g:
                pass


def _mha_body(tc, xq, xk, xv, wq, wk, wv, wo, bq, bk, out):
    nc = tc.nc
    from contextlib import ExitStack

    with ExitStack() as ctx:
        singles = ctx.enter_context(tc.tile_pool(name="singles", bufs=1))
        persist = ctx.enter_context(tc.tile_pool(name="persist", bufs=1))
        ps_s = ctx.enter_context(tc.tile_pool(name="ps_s", bufs=2, space="PSUM"))
        ps_a = ctx.enter_context(tc.tile_pool(name="ps_a", bufs=2, space="PSUM"))
        ps_w = ctx.enter_context(tc.tile_pool(name="ps_w", bufs=2, space="PSUM"))

        ones_col = singles.tile([128, 1], BF16)
        nc.vector.memset(ones_col, 1.0)
        ones_row = singles.tile([128, 64], BF16)
        nc.vector.memset(ones_row, 1.0)
        ones_512 = singles.tile([1, 512], BF16)
        nc.vector.memset(ones_512, 1.0)
        bqr = singles.tile([1, 512], BF16)
        bkr = singles.tile([1, 512], BF16)

        qT = [persist.tile([128, S], BF16, name=f"qT{c}") for c in range(EC)]
        kT = [persist.tile([128, S], BF16, name=f"kT{c}") for c in range(EC)]
        v_sb = [persist.tile([128, H * DK], BF16, name=f"v{s}") for s in range(SM)]
        aT = [persist.tile([128, S], BF16, name=f"aT{p}") for p in range(EC)]
        wo_sb = []

        x_pool = ctx.enter_context(tc.tile_pool(name="x_sb", bufs=1))
        xq_sb = [x_pool.tile([128, S], BF16, name=f"xq{d}") for d in range(KD)]
        wq_sb = [x_pool.tile([128, E], BF16, name=f"wq{d}") for d in range(KD)]
        e_pool = ctx.enter_context(tc.tile_pool(name="e_sb", bufs=3))
        acc_pool = ctx.enter_context(tc.tile_pool(name="acc_sb", bufs=2))
        acc2_pool = ctx.enter_context(tc.tile_pool(name="acc2_sb", bufs=2))
        acc3_pool = ctx.enter_context(tc.tile_pool(name="acc3_sb", bufs=2))
        nrm_pool = ctx.enter_context(tc.tile_pool(name="nrm", bufs=1))
        late_pools = {}

        kv_ctx = ExitStack()
        kv_pool = kv_ctx.enter_context(tc.tile_pool(name="kv_sb", bufs=1))
        xk_sb = [kv_pool.tile([128, S], BF16, name=f"xk{d}") for d in range(KD)]
        xv_sb = [kv_pool.tile([128, S], BF16, name=f"xv{d}") for d in range(KD)]
        wk_sb = [kv_pool.tile([128, E], BF16, name=f"wk{d}") for d in range(KD)]
        wv_sb = [kv_pool.tile([128, E], BF16, name=f"wv{d}") for d in range(KD)]

        # --- input staging ---
        # Two parallel DMA paths: sync->HWDGE (~205GB/s, 625ns/instr hold)
        # and gpsimd->SWDGE (~123GB/s, ~1us Pool hold). One global
        # need-ordered chunk list, dispatched 2:1 sync:gpsimd. Nothing on
        # the scalar queue (ACT runs exp only); evac copies are on DVE.
        chunks = []

        def _add(sb_ap, dram_ap):
            chunks.append((sb_ap, dram_ap))

        for d in range(KD):
            _add(wk_sb[d], wk[d * 128 : (d + 1) * 128, :])
            _add(xk_sb[d][:, 0:512], xk[d * 128 : (d + 1) * 128, 0:512])
        for d in range(KD):
            _add(wq_sb[d], wq[d * 128 : (d + 1) * 128, :])
            _add(xq_sb[d][:, 0:512], xq[d * 128 : (d + 1) * 128, 0:512])
        for c in range(EC):
            _add(bkr[0:1, c * 128 : (c + 1) * 128], bk[c * 128 : (c + 1) * 128])
            _add(bqr[0:1, c * 128 : (c + 1) * 128], bq[c * 128 : (c + 1) * 128])
        for d in range(KD):
            _add(wv_sb[d], wv[d * 128 : (d + 1) * 128, :])
        for sc in range(SN):
            for d in range(KD):
                _add(
                    xv_sb[d][:, sc * 512 : (sc + 1) * 512],
                    xv[d * 128 : (d + 1) * 128, sc * 512 : (sc + 1) * 512],
                )
            if sc + 1 < SN:
                for d in range(KD):
                    _add(
                        xk_sb[d][:, (sc + 1) * 512 : (sc + 2) * 512],
                        xk[d * 128 : (d + 1) * 128, (sc + 1) * 512 : (sc + 2) * 512],
                    )
        for sc in range(1, SN):
            for d in range(KD):
                _add(
                    xq_sb[d][:, sc * 512 : (sc + 1) * 512],
                    xq[d * 128 : (d + 1) * 128, sc * 512 : (sc + 1) * 512],
                )
        for i, (sb_ap, dram_ap) in enumerate(chunks):
            q = nc.gpsimd if i % 3 == 2 else nc.sync
            q.dma_start(out=sb_ap, in_=dram_ap)
        def kproj_unit(p, s):
            p_ps = ps_w.tile([128, 512], F32, name="p_ps", tag="w")
            for d in range(KD):
                nc.tensor.matmul(
                    p_ps,
                    wk_sb[d][:, p * 128 : (p + 1) * 128],
                    xk_sb[d][:, s * 512 : (s + 1) * 512],
                    start=(d == 0),
                    stop=False,
                )
                yield 1
            nc.tensor.matmul(
                p_ps, bkr[0:1, p * 128 : (p + 1) * 128], ones_512[0:1, :],
                start=False, stop=True
            )
            yield 1
            nc.vector.tensor_copy(kT[p][:, s * 512 : (s + 1) * 512], p_ps)
            yield 0

        def qproj_unit(p, ic):
            p_ps = ps_w.tile([128, 512], F32, name="p_ps", tag="w")
            for d in range(KD):
                nc.tensor.matmul(
                    p_ps,
                    wq_sb[d][:, p * 128 : (p + 1) * 128],
                    xq_sb[d][:, ic * 512 : (ic + 1) * 512],
                    start=(d == 0),
                    stop=False,
                )
                yield 1
            nc.tensor.matmul(
                p_ps, bqr[0:1, p * 128 : (p + 1) * 128], ones_512[0:1, :],
                start=False, stop=True
            )
            yield 1
            nc.vector.tensor_copy(qT[p][:, ic * 512 : (ic + 1) * 512], p_ps)
            yield 0

        def outproj_unit(ic, s, e2):
            o_ps = ps_w.tile([128, 512], F32, name="o_ps", tag="w")
            for d in range(EC):
                nc.tensor.matmul(
                    o_ps,
                    aT[d][:, s * 128 : (s + 1) * 128],
                    wo_sb[d][:, e2 * 512 : (e2 + 1) * 512],
                    start=(d == 0),
                    stop=(d == EC - 1),
                )
                yield 1
            o_sb = late_pools["out"].tile([128, 512], F32, name="o_sb", tag="o")
            nc.vector.tensor_copy(o_sb, o_ps)
            nc.sync.dma_start(
                out=out[s * 128 : (s + 1) * 128, e2 * 512 : (e2 + 1) * 512],
                in_=o_sb,
            )
            yield 0

        def vproj_pp(half, j):
            # project V columns for heads {4*half..4*half+3}, key-tile j
            v_ps = ps_w.tile([128, 256], F32, name="v_ps", tag="w")
            for d in range(KD):
                nc.tensor.matmul(
                    v_ps,
                    xv_sb[d][:, j * 128 : (j + 1) * 128],
                    wv_sb[d][:, half * 256 : (half + 1) * 256],
                    start=(d == 0),
                    stop=(d == KD - 1),
                )
                yield 1
            nc.vector.tensor_copy(v_sb[j][:, half * 256 : (half + 1) * 256], v_ps)
            yield 0

        pump = _Pump()

        def make_tail(p, ic, a_ps, acc_ev, acc_od, acc_p, e_last):
            # trailing work of block (p, ic), split in two: tail_pe at j==0
            # of the next block (last attnV pair + final odd-chain add, so
            # e_last's buffer recycles quickly), tail_fin at j==2 (E_sum
            # merge + denominators + normalization, off the e-recycle path).
            i0 = ic * 512
            h0, h1 = 2 * p, 2 * p + 1

            def tail_pe():
                nc.tensor.matmul(
                    a_ps[0:64, :],
                    v_sb[SM - 1][:, h0 * DK : (h0 + 1) * DK],
                    e_last[:, 0:512],
                    start=False,
                    stop=True,
                )
                nc.tensor.matmul(
                    a_ps[64:128, :],
                    v_sb[SM - 1][:, h1 * DK : (h1 + 1) * DK],
                    e_last[:, 512:1024],
                    start=False,
                    stop=True,
                )
                with nc.allow_low_precision(
                    reason="E_sum bf16 accumulation; Z error averages out"
                ):
                    nc.vector.tensor_add(acc_od, acc_od, e_last)

            def tail_fin():
                with nc.allow_low_precision(
                    reason="E_sum bf16 accumulation; Z error averages out"
                ):
                    nc.vector.tensor_add(acc_ev, acc_ev, acc_od)
                    if acc_p is not None:
                        nc.vector.tensor_add(acc_ev, acc_ev, acc_p)
                acc = acc_ev
                # Z rows via column-tiled ones-matmul pair on E_sum
                z_ps = ps_w.tile([128, 512], F32, name="z_ps", tag="w")
                nc.tensor.matmul(
                    z_ps[0:1, :], ones_col, acc[:, 0:512], start=True, stop=True
                )
                nc.tensor.matmul(
                    z_ps[64:65, :], ones_col, acc[:, 512:1024],
                    start=True, stop=True,
                )
                rec = nrm_pool.tile([128, 512], BF16, name="rec", tag="rec")
                with nc.allow_low_precision(
                    reason="softmax denom reciprocal; bf16 err ~4e-3 verified"
                ):
                    nc.vector.reciprocal(rec[0:1, :], z_ps[0:1, :])
                    nc.vector.reciprocal(rec[64:65, :], z_ps[64:65, :])
                # broadcast 1/Z rows to 64 partitions each (K=1 pair)
                bc_ps = z_ps
                nc.tensor.matmul(
                    bc_ps[0:64, :], ones_row[0:1, :], rec[0:1, :],
                    start=True, stop=True,
                )
                nc.tensor.matmul(
                    bc_ps[64:128, :], ones_row[64:65, :], rec[64:65, :],
                    start=True, stop=True,
                )
                bc_sb = nrm_pool.tile([128, 512], BF16, name="bc_sb", tag="bc")
                with nc.allow_low_precision(
                    reason="softmax denom broadcast; bf16 err ~4e-3 verified"
                ):
                    nc.vector.tensor_copy(bc_sb, bc_ps)
                    nc.vector.tensor_mul(aT[p][:, i0 : i0 + 512], a_ps, bc_sb)

            return tail_pe, tail_fin

        def block(p, ic, prev_tail):
            i0 = ic * 512
            h0, h1 = 2 * p, 2 * p + 1
            pump.drain_due((p, ic))
            a_ps = ps_a.tile([128, 512], F32, name="a_ps", tag="a")
            acc_ev = acc_pool.tile([128, 1024], BF16, name="acc_ev", tag="acc")
            acc_od = acc2_pool.tile([128, 1024], BF16, name="acc_od", tag="acc2")
            acc_p = (
                acc3_pool.tile([128, 1024], BF16, name="acc_p", tag="acc3")
                if p > 0
                else None
            )
            e_tiles = {}
            for j in range(SM):
                s_ps = ps_s.tile([128, 1024], F32, name="s_ps", tag="s")
                with tc.high_priority():
                    nc.tensor.matmul(
                        s_ps[:, 0:512],
                        kT[p][0:64, j * 128 : (j + 1) * 128],
                        qT[p][0:64, i0 : i0 + 512],
                        start=True,
                        stop=True,
                    )
                    nc.tensor.matmul(
                        s_ps[:, 512:1024],
                        kT[p][64:128, j * 128 : (j + 1) * 128],
                        qT[p][64:128, i0 : i0 + 512],
                        start=True,
                        stop=True,
                    )
                ep = late_pools.get("e2", e_pool)
                e0 = ep.tile([128, 1024], BF16, name="e0", tag="e")
                nc.scalar.activation(e0, s_ps, EXP, scale=0.125)
                e_tiles[j] = e0
                if j == 0 and prev_tail is not None:
                    prev_tail[0]()
                if j == 4 and prev_tail is not None:
                    prev_tail[1]()
                    for due, gen in pending:
                        pump.push(due, gen)
                    del pending[:]
                if j > 0:
                    ej = e_tiles.pop(j - 1)
                    nc.tensor.matmul(
                        a_ps[0:64, :],
                        v_sb[j - 1][:, h0 * DK : (h0 + 1) * DK],
                        ej[:, 0:512],
                        start=(j - 1 == 0),
                        stop=False,
                    )
                    nc.tensor.matmul(
                        a_ps[64:128, :],
                        v_sb[j - 1][:, h1 * DK : (h1 + 1) * DK],
                        ej[:, 512:1024],
                        start=(j - 1 == 0),
                        stop=False,
                    )
                    jj = j - 1
                    if jj % 2 == 0:
                        eng, acc, first = nc.vector, acc_ev, jj == 0
                    elif acc_p is not None and jj % 4 == 3:
                        eng, acc, first = nc.gpsimd, acc_p, jj == 3
                    else:
                        eng, acc, first = nc.vector, acc_od, jj == 1
                    with nc.allow_low_precision(
                        reason="E_sum bf16 accumulation; Z error averages out"
                    ):
                        if first:
                            eng.tensor_copy(acc, ej)
                        else:
                            eng.tensor_add(acc, acc, ej)
                if p == 0 and ic == 0:
                    for _ in vproj_pp(0, j):
                        pass
                pump.pump(1 if (p == 0 and ic == 0) else (3 if pump.has_due((p, ic)) else 2), (p, ic))
            return make_tail(p, ic, a_ps, acc_ev, acc_od, acc_p, e_tiles.pop(SM - 1))

        # --- prologue: first two kproj chunks + first qproj, serial ---
        for _ in kproj_unit(0, 0):
            pass
        for _ in qproj_unit(0, 0):
            pass
        pump.push((0, -1), kproj_unit(0, 1))
        pump.push((0, -1), kproj_unit(0, 2))
        pump.push((0, -1), kproj_unit(0, 3))
        for ic in range(1, SN):
            pump.push((0, ic - 1.1), qproj_unit(0, ic))

        # --- main loop: ic-fast within head-pair groups ---
        tail = None
        pending = []
        for p in range(EC):
            if p + 1 < EC:
                # next group prereqs, spread across this group
                for sc in range(SN):
                    pump.push((p, sc - 0.5), kproj_unit(p + 1, sc))
                for ic in range(SN):
                    pump.push((p, ic - 0.5), qproj_unit(p + 1, ic))
            if p < 2:
                # vproj for heads 4-7 (pairs 2,3): spread over groups 0-1,
                # after each group's kproj/qproj prereqs in queue order
                for j in range(8 * p, 8 * p + 8):
                    pump.push((p, 2 + (j % 8) / 4.0), vproj_pp(1, j))
            for ic in range(SN):
                tail = block(p, ic, tail)
                if p == EC - 1:
                    for sq in range(4):
                        for e2 in range(2):
                            pending.append(
                                ((3, ic + sq / 4.0),
                                 outproj_unit(ic, ic * 4 + sq, e2))
                            )
            if p == 2:
                kv_ctx.close()
                late_pools["e2"] = ctx.enter_context(
                    tc.tile_pool(name="e2_sb", bufs=8)
                )
                late_pools["out"] = ctx.enter_context(
                    tc.tile_pool(name="osb", bufs=4)
                )
                late_pools["wo"] = ctx.enter_context(
                    tc.tile_pool(name="wo_sb", bufs=1)
                )
                for d in range(EC):
                    wo_sb.append(
                        late_pools["wo"].tile([128, D], BF16, name=f"wo{d}")
                    )
                    nc.sync.dma_start(
                        out=wo_sb[d], in_=wo[d * 128 : (d + 1) * 128, :]
                    )
        tail[0]()
        tail[1]()
        for due, gen in pending:
            pump.push(due, gen)
        del pending[:]
        pump.drain_all()


def _prep_in_maps(query, key, value, w_q, b_q, w_k, b_k, w_v, b_v, w_o):
    f32 = np.float32
    in_maps = []
    for c in range(NCORES):
        b, g = c // G, c % G
        sl = slice(g * E, (g + 1) * E)
        in_maps.append(
            {
                "xq_t": np.ascontiguousarray(query[b].T).astype(NPBF16),
                "xk_t": np.ascontiguousarray(key[b].T).astype(NPBF16),
                "xv_t": np.ascontiguousarray(value[b].T).astype(NPBF16),
                "wq_t": np.ascontiguousarray(w_q[sl, :].T).astype(NPBF16),
                "wk_t": np.ascontiguousarray(w_k[sl, :].T).astype(NPBF16),
                "wv_t": np.ascontiguousarray(w_v[sl, :].T).astype(NPBF16),
                "wo_t": np.ascontiguousarray(w_o[:, sl].T).astype(NPBF16),
                "b_q": np.ascontiguousarray(b_q[sl]).astype(NPBF16),
                "b_k": np.ascontiguousarray(b_k[sl]).astype(NPBF16),
            }
        )
    return in_maps


_NC_CACHE = {}


def run(inputs, trace=False, **kw):
    if REPS not in _NC_CACHE:
        _NC_CACHE[REPS] = _build_mha_nc(REPS)
    nc = _NC_CACHE[REPS]
    in_maps = _prep_in_maps(
        inputs["query"], inputs["key"], inputs["value"],
        inputs["w_q"], inputs["b_q"], inputs["w_k"], inputs["b_k"],
        inputs["w_v"], inputs["b_v"], inputs["w_o"],
    )
    res = run_bass_kernel_spmd(nc, in_maps, list(range(NCORES)), trace=trace, **kw)
    bias_vec = (
        np.asarray(inputs["b_o"], dtype=np.float32)
        + np.asarray(inputs["w_o"], dtype=np.float32)
        @ np.asarray(inputs["b_v"], dtype=np.float32)
    )
    full = np.empty((B, S, D), dtype=np.float32)
    for b in range(B):
        full[b] = res.results[2 * b]["out"] + res.results[2 * b + 1]["out"] + bias_vec
    return full, res


def kernel(**inputs):
    full, _ = run(inputs)
    return full


def _make_timed_callable(nc, in_maps):
    import jax
    from jax.sharding import Mesh, PartitionSpec
    from jax.experimental.shard_map import shard_map
    from concourse import bass2jax, mybir as mb

    partition_name = nc.partition_id_tensor.name if nc.partition_id_tensor else None
    in_names, out_names, out_avals, zero_outs = [], [], [], []
    for alloc in nc.m.functions[0].allocations:
        if not isinstance(alloc, mb.MemoryLocationSet):
            continue
        name = alloc.memorylocations[0].name
        if alloc.kind == "ExternalInput":
            if name != partition_name:
                in_names.append(name)
        elif alloc.kind == "ExternalOutput":
            out_names.append(name)
            shape = tuple(alloc.tensor_shape)
            dtype = mb.dt.np(alloc.dtype)
            out_avals.append(jax.core.ShapedArray(shape, dtype))
            zero_outs.append(np.zeros(shape, dtype))
    n_params = len(in_names)
    in_names = in_names + out_names
    if partition_name is not None:
        in_names.append(partition_name)
    donate = tuple(range(n_params, n_params + len(out_names)))

    def _body(*args):
        operands = list(args)
        if partition_name is not None:
            operands.append(bass2jax.partition_id_tensor())
        outs = bass2jax._bass_exec_p.bind(
            *operands,
            out_avals=tuple(out_avals),
            in_names=tuple(in_names),
            out_names=tuple(out_names),
            lowering_input_output_aliases=(),
            sim_require_finite=True,
            sim_require_nnan=True,
            nc=nc,
        )
        return tuple(outs)

    devices = jax.devices()[:NCORES]
    mesh = Mesh(np.asarray(devices).reshape(NCORES), ("core",))
    in_specs = (PartitionSpec("core"),) * (n_params + len(out_names))
    out_specs = (PartitionSpec("core"),) * len(out_names)
    sharded = jax.jit(
        shard_map(_body, mesh=mesh, in_specs=in_specs, out_specs=out_specs,
                  check_rep=False),
        donate_argnums=donate, keep_unused=True,
    )
    concat_in = [
        np.concatenate([in_maps[c][in_names[i]] for c in range(NCORES)], axis=0)
        for i in range(n_params)
    ]
    dev_in = [jax.device_put(a) for a in concat_in]

    def call():
        zeros_dev = [
            jax.device_put(np.zeros((NCORES * z.shape[0], *z.shape[1:]), z.dtype))
            for z in zero_outs
        ]
        jax.block_until_ready(zeros_dev)
        import time

        t0 = time.perf_counter()
        out_arrs = sharded(*dev_in, *zeros_dev)
        jax.block_until_ready(out_arrs)
        dt = time.perf_counter() - t0
        return out_arrs, dt

    def gather(out_arrs):
        return [
            {
                name: np.asarray(out_arrs[i]).reshape(NCORES, *out_avals[i].shape)[c]
                for i, name in enumerate(out_names)
            }
            for c in range(NCORES)
        ]

    return call, gather


def run_timed(inputs, iters=6):
    """Measure device execution via repeated pjrt calls (amortizes RPC)."""
    global REPS
    from concourse import bass2jax

    bass2jax.install_neuronx_cc_hook()
    in_maps = _prep_in_maps(
        inputs["query"], inputs["key"], inputs["value"],
        inputs["w_q"], inputs["b_q"], inputs["w_k"], inputs["b_k"],
        inputs["w_v"], inputs["b_v"], inputs["w_o"],
    )
    if REPS not in _NC_CACHE:
        _NC_CACHE[REPS] = _build_mha_nc(REPS)
    call, gather = _make_timed_callable(_NC_CACHE[REPS], in_maps)
    times = []
    out_arrs = None
    for _ in range(iters):
        out_arrs, dt = call()
        times.append(dt)
    res = gather(out_arrs)
    bias_vec = (
        np.asarray(inputs["b_o"], dtype=np.float32)
        + np.asarray(inputs["w_o"], dtype=np.float32)
        @ np.asarray(inputs["b_v"], dtype=np.float32)
    )
    full = np.empty((B, S, D), dtype=np.float32)
    for b in range(B):
        full[b] = res[2 * b]["out"] + res[2 * b + 1]["out"] + bias_vec
    return full, times


def run_timed_pair(inputs, nrep=25, iters=40):
    """Interleave REPS=1 and REPS=nrep executions so slow wall-clock drift
    cancels in the per-iteration delta. Returns (full_output_r1, deltas_s)
    where deltas[i] = t_rn[i] - t_r1[i]; kernel time ~= median(deltas)/(nrep-1).
    """
    global REPS
    from concourse import bass2jax

    bass2jax.install_neuronx_cc_hook()
    in_maps = _prep_in_maps(
        inputs["query"], inputs["key"], inputs["value"],
        inputs["w_q"], inputs["b_q"], inputs["w_k"], inputs["b_k"],
        inputs["w_v"], inputs["b_v"], inputs["w_o"],
    )
    old = REPS
    try:
        for r in (1, nrep):
            REPS = r
            if r not in _NC_CACHE:
                _NC_CACHE[r] = _build_mha_nc(r)
    finally:
        REPS = old
    call1, gather1 = _make_timed_callable(_NC_CACHE[1], in_maps)
    calln, gathern = _make_timed_callable(_NC_CACHE[nrep], in_maps)
    out1, _ = call1()
    outn, _ = calln()  # compile+warm both
    t1s, tns = [], []
    for _ in range(iters):
        out1, dt1 = call1()
        outn, dtn = calln()
        t1s.append(dt1)
        tns.append(dtn)
    res1, resn = gather1(out1), gathern(outn)
    for c in range(NCORES):
        assert np.allclose(res1[c]["out"], resn[c]["out"], atol=1e-5)
    bias_vec = (
        np.asarray(inputs["b_o"], dtype=np.float32)
        + np.asarray(inputs["w_o"], dtype=np.float32)
        @ np.asarray(inputs["b_v"], dtype=np.float32)
    )
    full = np.empty((B, S, D), dtype=np.float32)
    for b in range(B):
        full[b] = res1[2 * b]["out"] + res1[2 * b + 1]["out"] + bias_vec
    return full, t1s, tns
